# revision 1
# baseline (speedup 1.0000x reference)
"""Multi-head attention block (B=8, S=1024, H=768, 12 heads x 64) on 8 TRN2 cores.

Sharding: pure data-parallel - one batch element per NeuronCore, no collectives.

Per-core pipeline (v2, ACT-bound design):
  - bf16 weights/xT shipped from host; all projection/score matmuls in bf16.
  - QK chunk c -> scores for heads (2c, 2c+1) immediately -> exp on ACT starts
    ~12us in and stays saturated (exp is the critical engine: 96 x [128,1024]).
  - Score matmul pairs packed onto PE row-groups 0-63 / 64-127 via
    tile_position, so both heads' scores stream concurrently.
  - exp output in fp8e4 (stationary of context matmul -> 4x faster LDWEIGHTS
    via FWL); V stored fp8 with a 0.5-column so the context matmul's column 64
    yields sum(exp)/2 and the softmax division folds the DropPath x2.
  - Residual + 2*bv folded into xn host-side; LayerNorm split across GpSimd
    (residual add, per-row algebra), DVE (row-sum, normalize) and ACT (fused
    center+square+accumulate, sqrt - ACT is idle once the exps are done).

PSUM budget (8 banks): proj/score tiles [128,1024]x2 (4 banks) + context
tiles [128,1024]x2 viewed [128,4,256] (4 banks).

Notes pinned by hardware probes in this container: tensor_tensor_reduce and
tensor_scalar(accum_out=...) on DVE fail at runtime/verifier - use plain
tensor_reduce or ACT accum_out instead. fp8 matmul, GpSimd tensor ops, and
explicit tile_position all work.
"""

import sys

sys.path.insert(0, "/opt/trn_rl_repo")

import numpy as np
import ml_dtypes
from contextlib import ExitStack

import concourse.bacc as bacc
import concourse.tile as tile
from concourse import mybir
from concourse import bass_utils

AF = mybir.ActivationFunctionType
ALU = mybir.AluOpType
AX = mybir.AxisListType

import os

F32 = mybir.dt.float32
I16 = mybir.dt.int16
BF16 = mybir.dt.bfloat16
FP8 = mybir.dt.bfloat16 if os.environ.get("K_NO_FP8") else mybir.dt.float8e4
TILE_POS = not os.environ.get("K_NO_TILEPOS")
GPS = not os.environ.get("K_NO_GPSIMD")
# Schraudolph fast-exp offload to DVE+GpSimd: numerically verified on HW but
# measured SLOWER in an in-process A/B (GpSimd per-instruction overhead stalls
# the ctx pipeline) - keep opt-in for future experiments.
FEXP = bool(os.environ.get("K_FEXP"))
# software-pipelined projections: chunk c+1's Q/K projections run in ~2.6us
# quanta between chunk c's score groups, through the "cx" psum pool, so the
# "mm" pool stays a dedicated 2-tile score buffer for ACT during proj windows
PIPE = not os.environ.get("K_NO_PIPE")
# ctx trails the score pair by TRAIL pairs; ACT runs only ~2 score tiles
# behind PE, so trail-2 is safe and shrinks both the end tail and the pool
TRAIL = int(os.environ.get("K_TRAIL", "3"))
# PE warmup + ACT table preload: measured SLOWER in-process (median +20us,
# worse in 4/5 paired trials) - the prologue work delays the real pipeline
# more than the clock ramp / table loads cost. Opt-in for experiments.
WARM = bool(os.environ.get("K_WARM"))
# ACT table preloads alone (no PE warm MMs): both sit in provably idle
# windows of the in-order ACT queue - Exp table loads under the DMA
# prologue, Sqrt table under the post-exp ctx tail. ~2.7us each.
TPRE = not os.environ.get("K_NO_TPRE")
# PSUM rebalance (3-tile score buffer + single-buffered cx): measured
# SLOWER in-process (lost 5/5 paired trials, +34us median) - serializing
# every proj/V/ctx fill on its consumer outweighs the extra ACT buffer.
PSUM3 = bool(os.environ.get("K_PSUM3"))
MMB = 3 if PSUM3 else 2
CXB = 1 if PSUM3 else 2
# interleave the last head-pair's ctx halves with LayerNorm rows so the LN
# chain starts 4 rows early instead of waiting for the whole pair
# last-pair ctx/LN interleave: A/B inconclusive (effect below the ±30us
# trial noise; medians 137 vs 142us, paired diffs slightly the other way).
# Opt-in; default stays with the simpler proven tail.
TAILIL = bool(os.environ.get("K_TAILIL"))
OFFLOAD_J = (3, 6)
# bits = s * (0.125*log2e*128) + (127 - 2.5*0.125*log2e... folded) * 128 - sigma
FEXP_A = 23.083120654223414
FEXP_B = 15788.76

B, S, H, NH, DH = 8, 1024, 768, 12, 64
P = 128
HC = H // P   # 6 chunks of the feature dim
SC = S // P   # 8 chunks of the sequence dim
VW = NH * 65  # V storage width: 64 cols + 1 half-col per head
EPS = 1e-6

_cache = {}


def _build(affine: bool, repeats: int = 1):
    nc = bacc.Bacc("TRN2", target_bir_lowering=False, debug=False)

    xT_d = nc.dram_tensor("xT", [H, S], BF16, kind="ExternalInput")
    xn_d = nc.dram_tensor("xn", [S, H], F32, kind="ExternalInput")
    wq_d = nc.dram_tensor("wq", [H, H], BF16, kind="ExternalInput")
    wk_d = nc.dram_tensor("wk", [H, H], BF16, kind="ExternalInput")
    wv_d = nc.dram_tensor("wv", [H, H], BF16, kind="ExternalInput")
    bq_d = nc.dram_tensor("bq", [H], F32, kind="ExternalInput")
    bk_d = nc.dram_tensor("bk", [H], F32, kind="ExternalInput")
    if affine:
        gam_d = nc.dram_tensor("gam", [H], F32, kind="ExternalInput")
        bet_d = nc.dram_tensor("bet", [H], F32, kind="ExternalInput")
    y_d = nc.dram_tensor("y", [S, H], F32, kind="ExternalOutput")

    dram = dict(xT_d=xT_d, xn_d=xn_d, wq_d=wq_d, wk_d=wk_d, wv_d=wv_d,
                bq_d=bq_d, bk_d=bk_d, y_d=y_d,
                gam_d=gam_d if affine else None,
                bet_d=bet_d if affine else None)
    with ExitStack() as stk:
        tc = stk.enter_context(tile.TileContext(nc))
        for rep in range(repeats):
            if rep:
                tc.strict_bb_all_engine_barrier()
            _emit_once(nc, tc, dram, affine, rep)
    nc.compile()
    return nc


def _emit_once(nc, tc, dram, affine, rep):
    xT_d, xn_d, y_d = dram["xT_d"], dram["xn_d"], dram["y_d"]
    wq_d, wk_d, wv_d = dram["wq_d"], dram["wk_d"], dram["wv_d"]
    bq_d, bk_d = dram["bq_d"], dram["bk_d"]
    gam_d, bet_d = dram["gam_d"], dram["bet_d"]
    with ExitStack() as stk:
        lp = stk.enter_context(tc.tile_pool(name=f"long{rep}", bufs=1))
        ap = stk.enter_context(tc.tile_pool(name=f"attn{rep}", bufs=1))
        ps = stk.enter_context(tc.tile_pool(name=f"ps{rep}", bufs=1, space="PSUM"))

        # ---- loads ----
        bq_sb = lp.tile([P, HC], F32, tag="bq")
        nc.sync.dma_start(bq_sb, bq_d[:].rearrange("(c p) -> p c", p=P))
        bk_sb = lp.tile([P, HC], F32, tag="bk")
        nc.sync.dma_start(bk_sb, bk_d[:].rearrange("(c p) -> p c", p=P))

        # DMA order = need order: wq+xT gate the first projection, then wk
        # (first scores), wv, then the residual rows (only needed at LN).
        def load_w(d):
            out = []
            for c in range(HC):
                t = lp.tile([P, H], BF16, tag=f"w{d.name}{c}", name=f"w{d.name}{c}")
                nc.sync.dma_start(t, d[c * P:(c + 1) * P, :])
                out.append(t)
            return out

        W = {"q": []}
        xT = []
        for c in range(HC):
            t = lp.tile([P, H], BF16, tag=f"wwq{c}", name=f"wwq{c}")
            nc.sync.dma_start(t, wq_d[c * P:(c + 1) * P, :])
            W["q"].append(t)
            t = lp.tile([P, S], BF16, tag=f"xt{c}", name=f"xt{c}")
            nc.sync.dma_start(t, xT_d[c * P:(c + 1) * P, :])
            xT.append(t)
        W["k"] = load_w(wk_d)
        W["v"] = load_w(wv_d)
        XS = []
        for m in range(SC):
            t = ap.tile([P, H], F32, tag=f"xs{m}", name=f"xs{m}")
            nc.sync.dma_start(t, xn_d[m * P:(m + 1) * P, :])
            XS.append(t)

        ones1 = lp.tile([1, P], F32, tag="ones1")
        nc.vector.memset(ones1, 1.0)
        eshift = lp.tile([P, 1], F32, tag="eshift")
        nc.vector.memset(eshift, -2.5)
        epsc = ap.tile([P, 1], F32, tag="epsc", bufs=1)
        nc.vector.memset(epsc, EPS)

        if TPRE:
            # preload the Exp activation table set during the DMA prologue so
            # the first real exp doesn't pay the ~2.7us table load
            tpre = ap.tile([P, 1], F32, tag="tpre", bufs=2)
            nc.scalar.activation(tpre, eshift, AF.Exp)
        if WARM:
            # PE warmup chain: keep the PE activity monitor busy through the
            # prologue so the first projections run at full clock
            wz = lp.tile([16, 512], BF16, tag="warm")
            nc.vector.memset(wz, 0.0)
            wp = ps.tile([P, 512], F32, tag="cx", bufs=CXB, name="warm_ps")
            for _ in range(16):
                nc.tensor.matmul(wp, lhsT=wz[:, 0:128], rhs=wz,
                                 start=True, stop=True)

        if affine:
            def bcast_row(d_ap, tag):
                row = lp.tile([1, H], F32, tag=f"{tag}row", name=f"{tag}row")
                nc.sync.dma_start(row, d_ap[:].rearrange("(o h) -> o h", o=1))
                pt = ps.tile([P, 1024], F32, tag="mm", bufs=MMB, name=f"bc{tag}")
                for ns, nn in ((0, 512), (512, 256)):
                    nc.tensor.matmul(
                        pt[:, ns:ns + nn],
                        lhsT=ones1,
                        rhs=row[:, ns:ns + nn],
                        start=True, stop=True,
                    )
                bc = lp.tile([P, H], F32, tag=f"{tag}bc", name=f"{tag}bc")
                nc.vector.tensor_copy(bc, pt[:, 0:H])
                return bc

            gambc = bcast_row(gam_d, "gam")
            betbc = bcast_row(bet_d, "bet")

        QT = [None] * HC
        KT = [None] * HC
        expT = [[None] * SC for _ in range(NH)]
        V = [None] * SC
        Y = [lp.tile([P, H], F32, tag=f"y{m}", name=f"y{m}") for m in range(SC)]

        def proj_qk_chunk(nm, b_sb, out_list, c):
            pt = ps.tile([P, 1024], F32, tag="cx" if PIPE else "mm", bufs=CXB,
                         name=f"p{nm}{c}")
            for ns in (0, 512):
                for k in range(HC):
                    nc.tensor.matmul(
                        pt[:, ns:ns + 512],
                        lhsT=W[nm][k][:, c * P:(c + 1) * P],
                        rhs=xT[k][:, ns:ns + 512],
                        start=(k == 0), stop=(k == HC - 1),
                    )
            t = lp.tile([P, S], BF16, tag=f"{nm}t{c}", name=f"{nm}t{c}")
            nc.vector.tensor_scalar(
                out=t, in0=pt, scalar1=b_sb[:, c:c + 1], scalar2=None,
                op0=ALU.add,
            )
            out_list[c] = t

        def emit_scores_pair(c, jr=None):
            # heads (2c, 2c+1): row-groups 0-63 / 64-127 run concurrently
            for j in (jr if jr is not None else range(SC)):
                pe = ps.tile([P, S], F32, tag="mm", bufs=MMB, name=f"se{c}_{j}")
                po = ps.tile([P, S], F32, tag="mm", bufs=MMB, name=f"so{c}_{j}")
                for ns in (0, 512):
                    nc.tensor.matmul(
                        pe[:, ns:ns + 512],
                        lhsT=KT[c][0:64, j * P:(j + 1) * P],
                        rhs=QT[c][0:64, ns:ns + 512],
                        start=True, stop=True,
                        tile_position=(0, 0) if TILE_POS else None,
                    )
                    nc.tensor.matmul(
                        po[:, ns:ns + 512],
                        lhsT=KT[c][64:128, j * P:(j + 1) * P],
                        rhs=QT[c][64:128, ns:ns + 512],
                        start=True, stop=True,
                        tile_position=(64, 0) if TILE_POS else None,
                    )
                for h, pt in ((2 * c, pe), (2 * c + 1, po)):
                    if FEXP and j in OFFLOAD_J:
                        # Schraudolph fast-exp on DVE+GpSimd, offloading the
                        # bottleneck ACT engine: bits = s*A + B maps the score
                        # linearly into the bf16 exponent field; round to int16
                        # and reinterpret as bf16 (~2% rel err, diluted ~10x by
                        # the residual). Same exp(s/8 - 2.5) as the ACT path so
                        # the softmax denominator stays consistent.
                        sf = ap.tile([P, S], F32, tag="fexp", bufs=2,
                                     name=f"sf{h}_{j}")
                        nc.vector.tensor_scalar(
                            out=sf, in0=pt, scalar1=FEXP_A, scalar2=FEXP_B,
                            op0=ALU.mult, op1=ALU.add,
                        )
                        et = ap.tile([P, S], BF16, tag="expw", bufs=16,
                                     name=f"e{h}_{j}")
                        nc.gpsimd.tensor_copy(et[:, :].bitcast(I16), sf)
                    else:
                        et = ap.tile([P, S], FP8, tag="expt",
                                     bufs=16 * (TRAIL + 1) - 8,
                                     name=f"e{h}_{j}")
                        # constant shift keeps exp inside fp8e4m3 range
                        # (softmax is shift-invariant; the ones-column
                        # denominator rescales identically)
                        nc.scalar.activation(et, pt, AF.Exp,
                                             scale=1.0 / np.sqrt(DH),
                                             bias=eshift[:, 0:1])
                    expT[h][j] = et

        def emit_v(j):
            pt = ps.tile([P, 1024], F32, tag="cx" if PIPE else "mm", bufs=CXB,
                         name=f"pv{j}")
            for ns, nn in ((0, 512), (512, 256)):
                for k in range(HC):
                    nc.tensor.matmul(
                        pt[:, ns:ns + nn],
                        lhsT=xT[k][:, j * P:(j + 1) * P],
                        rhs=W["v"][k][:, ns:ns + nn],
                        start=(k == 0), stop=(k == HC - 1),
                    )
            vt = lp.tile([P, VW], FP8, tag=f"v{j}", name=f"v{j}")
            v3 = vt.rearrange("p (h d) -> p h d", d=65)
            nc.vector.tensor_copy(
                v3[:, :, 0:64],
                pt[:, 0:H].rearrange("p (h d) -> p h d", d=64),
            )
            # 0.5 ones-column: psum col 64 = sum(exp)/2, so its reciprocal is
            # 2/sum(exp) - the softmax division and the DropPath 2x in one
            (nc.gpsimd if GPS else nc.vector).memset(v3[:, :, 64:65], 0.5)
            V[j] = vt

        def emit_ctx_half(h, half):
            off = h * 65
            pc = ps.tile([P, 1024], F32, tag="cx", bufs=CXB, name=f"c{h}_{half}")
            pc4 = pc.rearrange("p (m d) -> p m d", d=256)
            for mi in range(4):
                m = half * 4 + mi
                for j in range(SC):
                    nc.tensor.matmul(
                        pc4[:, mi, 0:65],
                        lhsT=expT[h][j][:, m * P:(m + 1) * P],
                        rhs=V[j][:, off:off + 65],
                        start=(j == 0), stop=(j == SC - 1),
                    )
            rb = ap.tile([P, 4], F32, tag="rb", bufs=4, name=f"r{h}_{half}")
            nc.vector.reciprocal(rb, pc4[:, :, 64])
            for mi in range(4):
                m = half * 4 + mi
                nc.vector.tensor_scalar(
                    out=Y[m][:, h * 64:(h + 1) * 64], in0=pc4[:, mi, 0:64],
                    scalar1=rb[:, mi:mi + 1], scalar2=None, op0=ALU.mult,
                )

        def emit_ctx_head(h):
            for half in range(2):
                emit_ctx_half(h, half)
            for j in range(SC):
                expT[h][j] = None

        # ---- emission schedule ----
        # Score pair c is emitted in j-groups; exp-independent PE work (next
        # chunk's projections when PIPE, V, trailing ctx) fills the slots
        # between groups so ACT never drains its 2-tile psum score buffer.
        # ctx heads (pair c-3) interleave so expt-pool frees land mid-pair.
        V_PLAN = {1: ((0, 1), (2, 3), (4, 5))}
        proj_qk_chunk("q", bq_sb, QT, 0)
        proj_qk_chunk("k", bk_sb, KT, 0)
        for c in range(HC):
            if not PIPE and c > 0:
                proj_qk_chunk("q", bq_sb, QT, c)
                proj_qk_chunk("k", bk_sb, KT, c)
            if c == TRAIL:
                emit_v(6)
                emit_v(7)
            vs_ = V_PLAN.get(c, ())
            hpair = 2 * (c - TRAIL) if c >= TRAIL else None
            emit_scores_pair(c, range(0, 2))
            if PIPE and c + 1 < HC:
                proj_qk_chunk("q", bq_sb, QT, c + 1)
            if vs_:
                for vj in vs_[0]:
                    emit_v(vj)
            if hpair is not None:
                emit_ctx_head(hpair)
            emit_scores_pair(c, range(2, 5))
            if PIPE and c + 1 < HC:
                proj_qk_chunk("k", bk_sb, KT, c + 1)
            if vs_:
                for vj in vs_[1]:
                    emit_v(vj)
            if hpair is not None:
                emit_ctx_head(hpair + 1)
            emit_scores_pair(c, range(5, 8))
            if vs_:
                for vj in vs_[2]:
                    emit_v(vj)
        for h in range(2 * (HC - TRAIL), NH - 2 if TAILIL else NH):
            emit_ctx_head(h)

        # ---- residual + layernorm (overlaps context tail) ----
        if TPRE:
            # switch the ACT table set to sqrt's during the post-exp idle gap
            # instead of serializing the first LN row on the ~2.7us load
            tpre2 = ap.tile([P, 1], F32, tag="tpre", bufs=2)
            nc.scalar.activation(tpre2, epsc, AF.Sqrt)

        def ln_row(m):
            # residual add on GpSimd (frees DVE for the stats)
            (nc.gpsimd if GPS else nc.vector).tensor_tensor(
                out=Y[m], in0=Y[m], in1=XS[m], op=ALU.add)
            sm = ap.tile([P, 1], F32, tag="sm", bufs=3)
            nc.vector.tensor_reduce(out=sm, in_=Y[m], axis=AX.X, op=ALU.add)
            nm_t = ap.tile([P, 1], F32, tag="nm", bufs=3)
            (nc.gpsimd if GPS else nc.vector).tensor_scalar(
                out=nm_t, in0=sm, scalar1=-1.0 / H, scalar2=None, op0=ALU.mult
            )
            # fused center+square+row-sum on ACT (idle after the exps):
            # Square(y + (-mean)), accumulated; XS[m] is dead -> scratch out
            vs = ap.tile([P, 1], F32, tag="vs", bufs=3)
            nc.scalar.activation(XS[m], Y[m], AF.Square,
                                 bias=nm_t[:, 0:1], accum_out=vs)
            sd = ap.tile([P, 1], F32, tag="sd", bufs=3)
            nc.scalar.activation(sd, vs, AF.Sqrt,
                                 scale=1.0 / H, bias=epsc[:, 0:1])
            rstd = ap.tile([P, 1], F32, tag="rstd", bufs=3)
            nc.vector.reciprocal(rstd, sd)
            nc.vector.tensor_scalar(
                out=Y[m], in0=Y[m], scalar1=nm_t, scalar2=rstd,
                op0=ALU.add, op1=ALU.mult,
            )
            if affine:
                (nc.gpsimd if GPS else nc.vector).tensor_tensor(out=Y[m], in0=Y[m], in1=gambc, op=ALU.mult)
                (nc.gpsimd if GPS else nc.vector).tensor_tensor(out=Y[m], in0=Y[m], in1=betbc, op=ALU.add)
            nc.sync.dma_start(y_d[m * P:(m + 1) * P, :], Y[m])

        if TAILIL:
            for half in range(2):
                emit_ctx_half(NH - 2, half)
                emit_ctx_half(NH - 1, half)
                for mi in range(4):
                    ln_row(half * 4 + mi)
        else:
            for m in range(SC):
                ln_row(m)


def _get_nc(affine: bool):
    if affine not in _cache:
        _cache[affine] = _build(affine)
    return _cache[affine]


def _is_affine(inputs):
    gam = np.asarray(inputs["ln_gamma"], dtype=np.float32)
    bet = np.asarray(inputs["ln_beta"], dtype=np.float32)
    return not (np.all(gam == 1.0) and np.all(bet == 0.0))


def make_in_maps(inputs):
    x = np.asarray(inputs["x"], dtype=np.float32)
    Wq = np.asarray(inputs["Wq"], dtype=np.float32)
    Wk = np.asarray(inputs["Wk"], dtype=np.float32)
    Wv = np.asarray(inputs["Wv"], dtype=np.float32)
    bq = np.ascontiguousarray(np.asarray(inputs["bq"], dtype=np.float32))
    bk = np.ascontiguousarray(np.asarray(inputs["bk"], dtype=np.float32))
    bv = np.asarray(inputs["bv"], dtype=np.float32)
    affine = _is_affine(inputs)

    bf = ml_dtypes.bfloat16
    wq_b = np.ascontiguousarray(Wq.astype(bf))
    wk_b = np.ascontiguousarray(Wk.astype(bf))
    wv_b = np.ascontiguousarray(Wv.astype(bf))

    in_maps = []
    for b in range(B):
        im = {
            "xT": np.ascontiguousarray(x[b].T.astype(bf)),
            "xn": np.ascontiguousarray(x[b] + 2.0 * bv),
            "wq": wq_b, "wk": wk_b, "wv": wv_b,
            "bq": bq, "bk": bk,
        }
        if affine:
            im["gam"] = np.ascontiguousarray(
                np.asarray(inputs["ln_gamma"], dtype=np.float32))
            im["bet"] = np.ascontiguousarray(
                np.asarray(inputs["ln_beta"], dtype=np.float32))
        in_maps.append(im)
    return in_maps


def run(inputs, trace=False):
    nc = _get_nc(_is_affine(inputs))
    in_maps = make_in_maps(inputs)
    res = bass_utils.run_bass_kernel_spmd(
        nc, in_maps, core_ids=list(range(B)), trace=trace
    )
    out = np.stack([r["y"] for r in res.results], axis=0)
    return out, res


def kernel(**inputs) -> np.ndarray:
    out, _ = run(inputs, trace=False)
    return out



# revision 40
# speedup vs baseline: 14.5349x; 14.5349x over previous
"""Multi-head attention block (B=8, S=1024, H=768, 12 heads x 64) on 8 TRN2 cores.

Sharding: pure data-parallel - one batch element per NeuronCore, no collectives.

Per-core pipeline (v2, ACT-bound design):
  - bf16 weights/xT shipped from host; all projection/score matmuls in bf16.
  - QK chunk c -> scores for heads (2c, 2c+1) immediately -> exp on ACT starts
    ~12us in and stays saturated (exp is the critical engine: 96 x [128,1024]).
  - Score matmul pairs packed onto PE row-groups 0-63 / 64-127 via
    tile_position, so both heads' scores stream concurrently.
  - exp output in fp8e4 (stationary of context matmul -> 4x faster LDWEIGHTS
    via FWL); V stored fp8 with a 0.5-column so the context matmul's column 64
    yields sum(exp)/2 and the softmax division folds the DropPath x2.
  - Residual + 2*bv folded into xn host-side; LayerNorm split across GpSimd
    (residual add, per-row algebra), DVE (row-sum, normalize) and ACT (fused
    center+square+accumulate, sqrt - ACT is idle once the exps are done).

PSUM budget (8 banks): proj/score tiles [128,1024]x2 (4 banks) + context
tiles [128,1024]x2 viewed [128,4,256] (4 banks).

Notes pinned by hardware probes in this container: tensor_tensor_reduce and
tensor_scalar(accum_out=...) on DVE fail at runtime/verifier - use plain
tensor_reduce or ACT accum_out instead. fp8 matmul, GpSimd tensor ops, and
explicit tile_position all work.
"""

import sys

sys.path.insert(0, "/opt/trn_rl_repo")

import numpy as np
import ml_dtypes
from contextlib import ExitStack

import concourse.bacc as bacc
import concourse.tile as tile
from concourse import mybir
from concourse import bass_utils

AF = mybir.ActivationFunctionType
ALU = mybir.AluOpType
AX = mybir.AxisListType

import os

F32 = mybir.dt.float32
I16 = mybir.dt.int16
BF16 = mybir.dt.bfloat16
FP8 = mybir.dt.bfloat16 if os.environ.get("K_NO_FP8") else mybir.dt.float8e4
TILE_POS = not os.environ.get("K_NO_TILEPOS")
GPS = not os.environ.get("K_NO_GPSIMD")
# Schraudolph fast-exp on DVE in ONE tensor_scalar (f32 psum -> i16 sbuf,
# bitcast bf16): offloads the bottleneck ACT engine. X tiles per chunk
# (of 16) move to DVE. HW-probed: rel err ~2% per exp value, diluted ~10x
# by softmax normalization + residual + LN.
XPC = int(os.environ.get("K_XPC", "4"))
# offload pattern per x-count: j-sets for pe (head 2c) and po (head 2c+1)
_OFF_PAT = {
    0: ((), ()),
    1: ((), (4,)),
    2: ((), (2, 6)),
    3: ((), (1, 4, 6)),
    4: ((3, 7), (1, 5)),
    5: ((2, 6), (0, 3, 5)),
    6: ((1, 3, 5), (0, 2, 6)),
    7: ((1, 3, 5), (0, 2, 4, 6)),
    8: ((1, 3, 5, 7), (0, 2, 4, 6)),
}
# merge the 96 [128,64] ctx-normalize tensor_scalars into 24 broadcast
# tensor_tensors [128,4,64] (stride-0 in1) - frees ~13us of DVE
CTXM = not os.environ.get("K_NO_CTXM")
# V projection via fp8 DoubleRow (2 MACs/cell/cycle, k-chunks paired):
# halves V-proj matmul time. x/Wv in fp8 is accuracy-safe for the V path
# (V output is fp8-quantized anyway; errors dilute via attention averaging)
VDR = not os.environ.get("K_NO_VDR")
# split the first-projection-gating DMAs (wq/wk/xT) into column halves,
# first halves issued first: the c=0 projections only need the first
# cols, so PE starts ~4us earlier
DSPLIT = not os.environ.get("K_NO_DSPLIT")
# emit the Q/K psum->sbuf bias-add copies on ACT (activation Copy with
# per-row bias, no table switch) instead of DVE: frees DVE for exp
# offload and decouples the cx-pool recycle from the DVE queue
QKACT = bool(os.environ.get("K_QKACT"))
# offload count for the last chunk (tail balance); defaults to XPC
XPCL = int(os.environ.get("K_XPCL", str(XPC)))
# bf16 Y/residual/output: 2x DVE throughput on the LN tail ops, halves
# the xn/y DMA traffic. ~0.4% rms extra error (budget 2e-2)
YBF = not os.environ.get("K_NO_YBF16")
YDT = mybir.dt.bfloat16 if YBF else mybir.dt.float32
# fp8 DoubleRow for the Q/K projections too (reuses xt8): cuts the
# largest PE item (QK proj 30.7us -> ~18us). Adds ~5% rms error to q/k
# -> ~0.5% on the output; measure rel err before adopting.
QKDR = bool(os.environ.get("K_QKDR"))
assert VDR or not QKDR, "QKDR requires VDR (xt8/emit_v fp8 path)"
# software-pipelined projections: chunk c+1's Q/K projections run in ~2.6us
# quanta between chunk c's score groups, through the "cx" psum pool, so the
# "mm" pool stays a dedicated 2-tile score buffer for ACT during proj windows
PIPE = not os.environ.get("K_NO_PIPE")
# ctx trails the score pair by TRAIL pairs; ACT runs only ~2 score tiles
# behind PE, so trail-2 is safe and shrinks both the end tail and the pool
TRAIL = int(os.environ.get("K_TRAIL", "2"))
# PE warmup + ACT table preload: measured SLOWER in-process (median +20us,
# worse in 4/5 paired trials) - the prologue work delays the real pipeline
# more than the clock ramp / table loads cost. Opt-in for experiments.
WARM = bool(os.environ.get("K_WARM"))
# ACT table preloads alone (no PE warm MMs): both sit in provably idle
# windows of the in-order ACT queue - Exp table loads under the DMA
# prologue, Sqrt table under the post-exp ctx tail. ~2.7us each.
TPRE = not os.environ.get("K_NO_TPRE")
# PSUM rebalance (3-tile score buffer + single-buffered cx): measured
# SLOWER in-process (lost 5/5 paired trials, +34us median) - serializing
# every proj/V/ctx fill on its consumer outweighs the extra ACT buffer.
PSUM3 = bool(os.environ.get("K_PSUM3"))
MMB = 3 if PSUM3 else 2
CXB = 1 if PSUM3 else 2
# interleave the last head-pair's ctx halves with LayerNorm rows so the LN
# chain starts 4 rows early instead of waiting for the whole pair
# last-pair ctx/LN interleave: A/B inconclusive (effect below the ±30us
# trial noise; medians 137 vs 142us, paired diffs slightly the other way).
# Opt-in; default stays with the simpler proven tail.
TAILIL = bool(os.environ.get("K_TAILIL"))
# bits = s * (0.125*log2e*128) + (127 - 2.5*0.125*log2e... folded) * 128 - sigma
FEXP_A = 23.083120654223414
FEXP_B = 15788.76

B, S, H, NH, DH = 8, 1024, 768, 12, 64
P = 128
HC = H // P   # 6 chunks of the feature dim
SC = S // P   # 8 chunks of the sequence dim
VW = NH * 65  # V storage width: 64 cols + 1 half-col per head
EPS = 1e-6

_cache = {}


def _build(affine: bool, repeats: int = 1):
    nc = bacc.Bacc("TRN2", target_bir_lowering=False, debug=False)

    xT_d = (None if QKDR else
            nc.dram_tensor("xT", [H, S], BF16, kind="ExternalInput"))
    if YBF:
        xn_d = nc.dram_tensor("xnb", [S, H], BF16, kind="ExternalInput")
    else:
        xn_d = nc.dram_tensor("xn", [S, H], F32, kind="ExternalInput")
    if QKDR:
        wq_d = wk_d = None
    else:
        wq_d = nc.dram_tensor("wq", [H, H], BF16, kind="ExternalInput")
        wk_d = nc.dram_tensor("wk", [H, H], BF16, kind="ExternalInput")
    if VDR:
        # [p, k, s]/[p, k, n] layouts: k-chunk pairs addressable as the
        # [K, 2, *] APs DoubleRow wants
        xt8_d = nc.dram_tensor("xt8", [P, HC * S], FP8, kind="ExternalInput")
        wv8_d = nc.dram_tensor("wv8", [P, HC * H], FP8, kind="ExternalInput")
        wv_d = None
    else:
        xt8_d = wv8_d = None
        wv_d = nc.dram_tensor("wv", [H, H], BF16, kind="ExternalInput")
    if QKDR:
        wq8_d = nc.dram_tensor("wq8", [P, HC * H], FP8, kind="ExternalInput")
        wk8_d = nc.dram_tensor("wk8", [P, HC * H], FP8, kind="ExternalInput")
    else:
        wq8_d = wk8_d = None
    bq_d = nc.dram_tensor("bq", [H], F32, kind="ExternalInput")
    bk_d = nc.dram_tensor("bk", [H], F32, kind="ExternalInput")
    if affine:
        gam_d = nc.dram_tensor("gam", [H], F32, kind="ExternalInput")
        bet_d = nc.dram_tensor("bet", [H], F32, kind="ExternalInput")
    y_d = nc.dram_tensor("yb" if YBF else "y", [S, H], YDT,
                         kind="ExternalOutput")

    dram = dict(xT_d=xT_d, xn_d=xn_d, wq_d=wq_d, wk_d=wk_d, wv_d=wv_d,
                xt8_d=xt8_d, wv8_d=wv8_d, wq8_d=wq8_d, wk8_d=wk8_d,
                bq_d=bq_d, bk_d=bk_d, y_d=y_d,
                gam_d=gam_d if affine else None,
                bet_d=bet_d if affine else None)
    with ExitStack() as stk:
        tc = stk.enter_context(tile.TileContext(nc))
        for rep in range(repeats):
            if rep:
                tc.strict_bb_all_engine_barrier()
            _emit_once(nc, tc, dram, affine, rep)
    nc.compile()
    return nc


def _emit_once(nc, tc, dram, affine, rep):
    xT_d, xn_d, y_d = dram["xT_d"], dram["xn_d"], dram["y_d"]
    wq_d, wk_d, wv_d = dram["wq_d"], dram["wk_d"], dram["wv_d"]
    xt8_d, wv8_d = dram["xt8_d"], dram["wv8_d"]
    bq_d, bk_d = dram["bq_d"], dram["bk_d"]
    gam_d, bet_d = dram["gam_d"], dram["bet_d"]
    with ExitStack() as stk:
        lp = stk.enter_context(tc.tile_pool(name=f"long{rep}", bufs=1))
        ap = stk.enter_context(tc.tile_pool(name=f"attn{rep}", bufs=1))
        ps = stk.enter_context(tc.tile_pool(name=f"ps{rep}", bufs=1, space="PSUM"))

        # ---- loads ----
        bq_sb = lp.tile([P, HC], F32, tag="bq")
        nc.sync.dma_start(bq_sb, bq_d[:].rearrange("(c p) -> p c", p=P))
        bk_sb = lp.tile([P, HC], F32, tag="bk")
        nc.sync.dma_start(bk_sb, bk_d[:].rearrange("(c p) -> p c", p=P))

        # DMA order = need order: wq+xT gate the first projection, then wk
        # (first scores), wv, then the residual rows (only needed at LN).
        def load_w(d):
            out = []
            for c in range(HC):
                t = lp.tile([P, H], BF16, tag=f"w{d.name}{c}", name=f"w{d.name}{c}")
                nc.sync.dma_start(t, d[c * P:(c + 1) * P, :])
                out.append(t)
            return out

        W = {"q": []}
        xT = []
        if QKDR:
            # fp8 DR weights; xT bf16 still needed for... nothing in the
            # projections (scores use QT/KT), so only load xt8 + w8s
            wq8_d, wk8_d = dram["wq8_d"], dram["wk8_d"]
            wq8_sb = lp.tile([P, HC * H], FP8, tag="wq8", name="wq8")
            wk8_sb = lp.tile([P, HC * H], FP8, tag="wk8", name="wk8")
            xt8_sb0 = lp.tile([P, HC * S], FP8, tag="xt8", name="xt8")
            for c in range(0, HC, 2):
                nc.sync.dma_start(wq8_sb[:, c * H:(c + 2) * H],
                                  wq8_d[:, c * H:(c + 2) * H])
                nc.sync.dma_start(xt8_sb0[:, c * S:(c + 2) * S],
                                  xt8_d[:, c * S:(c + 2) * S])
            for c in range(0, HC, 2):
                nc.sync.dma_start(wk8_sb[:, c * H:(c + 2) * H],
                                  wk8_d[:, c * H:(c + 2) * H])
            wq3 = wq8_sb[:, :].rearrange("p (k n) -> p k n", k=HC)
            wk3 = wk8_sb[:, :].rearrange("p (k n) -> p k n", k=HC)
            x3full = xt8_sb0[:, :].rearrange("p (k s) -> p k s", k=HC)
            W8 = {"q": wq3, "k": wk3}
        elif DSPLIT:
            # first halves land first so the c=0 projections (which read
            # wq cols 0:128 and stream xT ns=0 first) start ~4us earlier
            for c in range(HC):
                W["q"].append(lp.tile([P, H], BF16, tag=f"wwq{c}",
                                      name=f"wwq{c}"))
                xT.append(lp.tile([P, S], BF16, tag=f"xt{c}", name=f"xt{c}"))
            W["k"] = [lp.tile([P, H], BF16, tag=f"wwk{c}", name=f"wwk{c}")
                      for c in range(HC)]
            for c in range(HC):
                nc.sync.dma_start(W["q"][c][:, 0:H // 2],
                                  wq_d[c * P:(c + 1) * P, 0:H // 2])
                nc.sync.dma_start(xT[c][:, 0:S // 2],
                                  xT_d[c * P:(c + 1) * P, 0:S // 2])
            for c in range(HC):
                nc.sync.dma_start(W["k"][c][:, 0:H // 2],
                                  wk_d[c * P:(c + 1) * P, 0:H // 2])
                nc.sync.dma_start(xT[c][:, S // 2:S],
                                  xT_d[c * P:(c + 1) * P, S // 2:S])
            for c in range(HC):
                nc.sync.dma_start(W["q"][c][:, H // 2:H],
                                  wq_d[c * P:(c + 1) * P, H // 2:H])
                nc.sync.dma_start(W["k"][c][:, H // 2:H],
                                  wk_d[c * P:(c + 1) * P, H // 2:H])
        else:
            for c in range(HC):
                t = lp.tile([P, H], BF16, tag=f"wwq{c}", name=f"wwq{c}")
                nc.sync.dma_start(t, wq_d[c * P:(c + 1) * P, :])
                W["q"].append(t)
                t = lp.tile([P, S], BF16, tag=f"xt{c}", name=f"xt{c}")
                nc.sync.dma_start(t, xT_d[c * P:(c + 1) * P, :])
                xT.append(t)
            W["k"] = load_w(wk_d)
        if VDR:
            # chunked transfers: one monolithic dma serializes ~34us on a
            # single DMA engine; 6/3-way splits land in parallel
            if QKDR:
                xt8_sb = xt8_sb0
            else:
                xt8_sb = lp.tile([P, HC * S], FP8, tag="xt8", name="xt8")
                for c in range(HC):
                    nc.sync.dma_start(xt8_sb[:, c * S:(c + 1) * S],
                                      xt8_d[:, c * S:(c + 1) * S])
            wv8_sb = lp.tile([P, HC * H], FP8, tag="wv8", name="wv8")
            for c in range(0, HC, 2):
                nc.sync.dma_start(wv8_sb[:, c * H:(c + 2) * H],
                                  wv8_d[:, c * H:(c + 2) * H])
            x3 = xt8_sb[:, :].rearrange("p (k s) -> p k s", k=HC)
            w3 = wv8_sb[:, :].rearrange("p (k n) -> p k n", k=HC)
        else:
            W["v"] = load_w(wv_d)
        XS = []
        for m in range(SC):
            t = ap.tile([P, H], YDT, tag=f"xs{m}", name=f"xs{m}")
            nc.sync.dma_start(t, xn_d[m * P:(m + 1) * P, :])
            XS.append(t)

        ones1 = lp.tile([1, P], F32, tag="ones1")
        nc.vector.memset(ones1, 1.0)
        eshift = lp.tile([P, 1], F32, tag="eshift")
        nc.vector.memset(eshift, -2.5)
        epsc = ap.tile([P, 1], F32, tag="epsc", bufs=1)
        nc.vector.memset(epsc, EPS)

        if TPRE:
            # preload the Exp activation table set during the DMA prologue so
            # the first real exp doesn't pay the ~2.7us table load
            tpre = ap.tile([P, 1], F32, tag="tpre", bufs=2)
            nc.scalar.activation(tpre, eshift, AF.Exp)
        if WARM:
            # PE warmup chain: keep the PE activity monitor busy through the
            # prologue so the first projections run at full clock
            wz = lp.tile([16, 512], BF16, tag="warm")
            nc.vector.memset(wz, 0.0)
            wp = ps.tile([P, 512], F32, tag="cx", bufs=CXB, name="warm_ps")
            for _ in range(16):
                nc.tensor.matmul(wp, lhsT=wz[:, 0:128], rhs=wz,
                                 start=True, stop=True)

        if affine:
            def bcast_row(d_ap, tag):
                row = lp.tile([1, H], F32, tag=f"{tag}row", name=f"{tag}row")
                nc.sync.dma_start(row, d_ap[:].rearrange("(o h) -> o h", o=1))
                pt = ps.tile([P, 1024], F32, tag="mm", bufs=MMB, name=f"bc{tag}")
                for ns, nn in ((0, 512), (512, 256)):
                    nc.tensor.matmul(
                        pt[:, ns:ns + nn],
                        lhsT=ones1,
                        rhs=row[:, ns:ns + nn],
                        start=True, stop=True,
                    )
                bc = lp.tile([P, H], YDT, tag=f"{tag}bc", name=f"{tag}bc")
                nc.vector.tensor_copy(bc, pt[:, 0:H])
                return bc

            gambc = bcast_row(gam_d, "gam")
            betbc = bcast_row(bet_d, "bet")

        QT = [None] * HC
        KT = [None] * HC
        expT = [[None] * SC for _ in range(NH)]
        V = [None] * SC
        # one big Y tile: lets the merged ctx-normalize write 4 m-blocks
        # (stride 768) in a single DVE instruction
        Ybig = lp.tile([P, SC * H], YDT, tag="ybig", name="ybig")
        Y = [Ybig[:, m * H:(m + 1) * H] for m in range(SC)]
        # [p, m, h, d] view for the merged normalize out AP
        Y4 = Ybig[:, :].rearrange("p (m hh d) -> p m hh d", m=SC, hh=NH)

        pe_j, po_j = _OFF_PAT[XPC]
        n_off = len(pe_j) + len(po_j)
        expw_bufs = max(2, n_off * (TRAIL + 1))
        expt_bufs = max(8, (16 - n_off) * (TRAIL + 1) - 8)

        def proj_qk_chunk(nm, b_sb, out_list, c):
            pt = ps.tile([P, 1024], F32, tag="cx" if PIPE else "mm", bufs=CXB,
                         name=f"p{nm}{c}")
            for ns in (0, 512):
                if QKDR:
                    for kp in range(HC // 2):
                        nc.tensor.matmul(
                            pt[:, ns:ns + 512],
                            lhsT=W8[nm][:, 2 * kp:2 * kp + 2,
                                        c * P:(c + 1) * P],
                            rhs=x3full[:, 2 * kp:2 * kp + 2, ns:ns + 512],
                            start=(kp == 0), stop=(kp == HC // 2 - 1),
                            perf_mode=mybir.MatmulPerfMode.DoubleRow,
                        )
                else:
                    for k in range(HC):
                        nc.tensor.matmul(
                            pt[:, ns:ns + 512],
                            lhsT=W[nm][k][:, c * P:(c + 1) * P],
                            rhs=xT[k][:, ns:ns + 512],
                            start=(k == 0), stop=(k == HC - 1),
                        )
            t = lp.tile([P, S], BF16, tag=f"{nm}t{c}", name=f"{nm}t{c}")
            if QKACT:
                # ACT Copy with per-row bias: out = in + b. Copy is in every
                # activation table set, so no table-switch cost.
                nc.scalar.activation(t, pt, AF.Copy, bias=b_sb[:, c:c + 1])
            else:
                nc.vector.tensor_scalar(
                    out=t, in0=pt, scalar1=b_sb[:, c:c + 1], scalar2=None,
                    op0=ALU.add,
                )
            out_list[c] = t

        def emit_scores_pair(c, jr=None):
            # heads (2c, 2c+1): row-groups 0-63 / 64-127 run concurrently
            for j in (jr if jr is not None else range(SC)):
                pe = ps.tile([P, S], F32, tag="mm", bufs=MMB, name=f"se{c}_{j}")
                po = ps.tile([P, S], F32, tag="mm", bufs=MMB, name=f"so{c}_{j}")
                for ns in (0, 512):
                    nc.tensor.matmul(
                        pe[:, ns:ns + 512],
                        lhsT=KT[c][0:64, j * P:(j + 1) * P],
                        rhs=QT[c][0:64, ns:ns + 512],
                        start=True, stop=True,
                        tile_position=(0, 0) if TILE_POS else None,
                    )
                    nc.tensor.matmul(
                        po[:, ns:ns + 512],
                        lhsT=KT[c][64:128, j * P:(j + 1) * P],
                        rhs=QT[c][64:128, ns:ns + 512],
                        start=True, stop=True,
                        tile_position=(64, 0) if TILE_POS else None,
                    )
                for h, pt, offl in ((2 * c, pe, j in pe_j),
                                    (2 * c + 1, po, j in po_j)):
                    if offl:
                        # Schraudolph fast-exp in ONE DVE tensor_scalar:
                        # bits = s*A + B maps the score linearly into the
                        # bf16 exponent field; the i16-convert-on-write
                        # rounds, bitcast reinterprets as bf16. Same
                        # exp(s/8 - 2.5) as the ACT path so the softmax
                        # denominator stays consistent.
                        et = ap.tile([P, S], BF16, tag="expw",
                                     bufs=expw_bufs, name=f"e{h}_{j}")
                        nc.vector.tensor_scalar(
                            out=et[:, :].bitcast(I16), in0=pt,
                            scalar1=FEXP_A, scalar2=FEXP_B,
                            op0=ALU.mult, op1=ALU.add,
                        )
                    else:
                        et = ap.tile([P, S], FP8, tag="expt",
                                     bufs=expt_bufs,
                                     name=f"e{h}_{j}")
                        # constant shift keeps exp inside fp8e4m3 range
                        # (softmax is shift-invariant; the ones-column
                        # denominator rescales identically)
                        nc.scalar.activation(et, pt, AF.Exp,
                                             scale=1.0 / np.sqrt(DH),
                                             bias=eshift[:, 0:1])
                    expT[h][j] = et

        def emit_v(j):
            pt = ps.tile([P, 1024], F32, tag="cx" if PIPE else "mm", bufs=CXB,
                         name=f"pv{j}")
            for ns, nn in ((0, 512), (512, 256)):
                if VDR:
                    for kp in range(HC // 2):
                        nc.tensor.matmul(
                            pt[:, ns:ns + nn],
                            lhsT=x3[:, 2 * kp:2 * kp + 2, j * P:(j + 1) * P],
                            rhs=w3[:, 2 * kp:2 * kp + 2, ns:ns + nn],
                            start=(kp == 0), stop=(kp == HC // 2 - 1),
                            perf_mode=mybir.MatmulPerfMode.DoubleRow,
                        )
                else:
                    for k in range(HC):
                        nc.tensor.matmul(
                            pt[:, ns:ns + nn],
                            lhsT=xT[k][:, j * P:(j + 1) * P],
                            rhs=W["v"][k][:, ns:ns + nn],
                            start=(k == 0), stop=(k == HC - 1),
                        )
            vt = lp.tile([P, VW], FP8, tag=f"v{j}", name=f"v{j}")
            v3 = vt.rearrange("p (h d) -> p h d", d=65)
            nc.vector.tensor_copy(
                v3[:, :, 0:64],
                pt[:, 0:H].rearrange("p (h d) -> p h d", d=64),
            )
            # 0.5 ones-column: psum col 64 = sum(exp)/2, so its reciprocal is
            # 2/sum(exp) - the softmax division and the DropPath 2x in one
            (nc.gpsimd if GPS else nc.vector).memset(v3[:, :, 64:65], 0.5)
            V[j] = vt

        def emit_ctx_half(h, half):
            off = h * 65
            pc = ps.tile([P, 1024], F32, tag="cx", bufs=CXB, name=f"c{h}_{half}")
            pc4 = pc.rearrange("p (m d) -> p m d", d=256)
            for mi in range(4):
                m = half * 4 + mi
                for j in range(SC):
                    nc.tensor.matmul(
                        pc4[:, mi, 0:65],
                        lhsT=expT[h][j][:, m * P:(m + 1) * P],
                        rhs=V[j][:, off:off + 65],
                        start=(j == 0), stop=(j == SC - 1),
                    )
            rb = ap.tile([P, 4], F32, tag="rb", bufs=4, name=f"r{h}_{half}")
            nc.vector.reciprocal(rb, pc4[:, :, 64])
            if CTXM:
                # one broadcast multiply for all 4 m-blocks: out strides
                # across Ybig m-blocks, in1 broadcasts rb over the 64 cols
                nc.vector.tensor_tensor(
                    out=Y4[:, half * 4:(half + 1) * 4, h, :],
                    in0=pc4[:, :, 0:64],
                    in1=rb[:, :].unsqueeze(2).broadcast_to([P, 4, 64]),
                    op=ALU.mult,
                )
            else:
                for mi in range(4):
                    m = half * 4 + mi
                    nc.vector.tensor_scalar(
                        out=Y[m][:, h * 64:(h + 1) * 64], in0=pc4[:, mi, 0:64],
                        scalar1=rb[:, mi:mi + 1], scalar2=None, op0=ALU.mult,
                    )

        def emit_ctx_head(h):
            for half in range(2):
                emit_ctx_half(h, half)
            for j in range(SC):
                expT[h][j] = None

        # ---- emission schedule ----
        # Score pair c is emitted in j-groups; exp-independent PE work (next
        # chunk's projections when PIPE, V, trailing ctx) fills the slots
        # between groups so ACT never drains its 2-tile psum score buffer.
        # ctx heads (pair c-3) interleave so expt-pool frees land mid-pair.
        if TRAIL >= 2:
            V_PLAN = {1: ((0, 1), (2, 3), (4, 5))}
        else:
            # ctx pair 0 runs during chunk 1 and needs ALL of V, so the V
            # projections must finish inside chunk 0
            V_PLAN = {0: ((0, 1), (2, 3), (4, 5))}
        proj_qk_chunk("q", bq_sb, QT, 0)
        proj_qk_chunk("k", bk_sb, KT, 0)
        for c in range(HC):
            if not PIPE and c > 0:
                proj_qk_chunk("q", bq_sb, QT, c)
                proj_qk_chunk("k", bk_sb, KT, c)
            if c == TRAIL:
                emit_v(6)
                emit_v(7)
            vs_ = V_PLAN.get(c, ())
            hpair = 2 * (c - TRAIL) if c >= TRAIL else None
            emit_scores_pair(c, range(0, 2))
            if PIPE and c + 1 < HC:
                proj_qk_chunk("q", bq_sb, QT, c + 1)
            if vs_:
                for vj in vs_[0]:
                    emit_v(vj)
            if hpair is not None:
                emit_ctx_head(hpair)
            emit_scores_pair(c, range(2, 5))
            if PIPE and c + 1 < HC:
                proj_qk_chunk("k", bk_sb, KT, c + 1)
            if vs_:
                for vj in vs_[1]:
                    emit_v(vj)
            if hpair is not None:
                emit_ctx_head(hpair + 1)
            emit_scores_pair(c, range(5, 8))
            if vs_:
                for vj in vs_[2]:
                    emit_v(vj)
        for h in range(2 * (HC - TRAIL), NH - 2 if TAILIL else NH):
            emit_ctx_head(h)

        # ---- residual + layernorm (overlaps context tail) ----
        if TPRE:
            # switch the ACT table set to sqrt's during the post-exp idle gap
            # instead of serializing the first LN row on the ~2.7us load
            tpre2 = ap.tile([P, 1], F32, tag="tpre", bufs=2)
            nc.scalar.activation(tpre2, epsc, AF.Sqrt)

        def ln_row(m):
            # residual add: with bf16 Y the DVE does this at 2x (~0.5us vs
            # 1.8us GpSimd), so only give GpSimd a minority of rows
            add_eng = nc.gpsimd if (GPS and (not YBF or m % 4 == 3)) else nc.vector
            add_eng.tensor_tensor(
                out=Y[m], in0=Y[m], in1=XS[m], op=ALU.add)
            sm = ap.tile([P, 1], F32, tag="sm", bufs=3)
            nc.vector.tensor_reduce(out=sm, in_=Y[m], axis=AX.X, op=ALU.add)
            nm_t = ap.tile([P, 1], F32, tag="nm", bufs=3)
            (nc.gpsimd if GPS else nc.vector).tensor_scalar(
                out=nm_t, in0=sm, scalar1=-1.0 / H, scalar2=None, op0=ALU.mult
            )
            # fused center+square+row-sum on ACT (idle after the exps):
            # Square(y + (-mean)), accumulated; XS[m] is dead -> scratch out
            vs = ap.tile([P, 1], F32, tag="vs", bufs=3)
            nc.scalar.activation(XS[m], Y[m], AF.Square,
                                 bias=nm_t[:, 0:1], accum_out=vs)
            sd = ap.tile([P, 1], F32, tag="sd", bufs=3)
            nc.scalar.activation(sd, vs, AF.Sqrt,
                                 scale=1.0 / H, bias=epsc[:, 0:1])
            rstd = ap.tile([P, 1], F32, tag="rstd", bufs=3)
            nc.vector.reciprocal(rstd, sd)
            nc.vector.tensor_scalar(
                out=Y[m], in0=Y[m], scalar1=nm_t, scalar2=rstd,
                op0=ALU.add, op1=ALU.mult,
            )
            if affine:
                (nc.gpsimd if GPS else nc.vector).tensor_tensor(out=Y[m], in0=Y[m], in1=gambc, op=ALU.mult)
                (nc.gpsimd if GPS else nc.vector).tensor_tensor(out=Y[m], in0=Y[m], in1=betbc, op=ALU.add)
            nc.sync.dma_start(y_d[m * P:(m + 1) * P, :], Y[m])

        if TAILIL:
            for half in range(2):
                emit_ctx_half(NH - 2, half)
                emit_ctx_half(NH - 1, half)
                for mi in range(4):
                    ln_row(half * 4 + mi)
        else:
            for m in range(SC):
                ln_row(m)


def _get_nc(affine: bool):
    if affine not in _cache:
        _cache[affine] = _build(affine)
    return _cache[affine]


def _is_affine(inputs):
    gam = np.asarray(inputs["ln_gamma"], dtype=np.float32)
    bet = np.asarray(inputs["ln_beta"], dtype=np.float32)
    return not (np.all(gam == 1.0) and np.all(bet == 0.0))


def make_in_maps(inputs):
    x = np.asarray(inputs["x"], dtype=np.float32)
    Wq = np.asarray(inputs["Wq"], dtype=np.float32)
    Wk = np.asarray(inputs["Wk"], dtype=np.float32)
    Wv = np.asarray(inputs["Wv"], dtype=np.float32)
    bq = np.ascontiguousarray(np.asarray(inputs["bq"], dtype=np.float32))
    bk = np.ascontiguousarray(np.asarray(inputs["bk"], dtype=np.float32))
    bv = np.asarray(inputs["bv"], dtype=np.float32)
    affine = _is_affine(inputs)

    bf = ml_dtypes.bfloat16
    f8 = ml_dtypes.float8_e4m3

    def pkn(Wm):
        return np.ascontiguousarray(
            Wm.reshape(HC, P, H).transpose(1, 0, 2).reshape(P, HC * H)
            .astype(f8))

    if QKDR:
        wq8 = pkn(Wq)
        wk8 = pkn(Wk)
    else:
        wq_b = np.ascontiguousarray(Wq.astype(bf))
        wk_b = np.ascontiguousarray(Wk.astype(bf))
    if VDR:
        # wv8[p, k, n] = Wv[k*128+p, n]
        wv8 = np.ascontiguousarray(
            Wv.reshape(HC, P, H).transpose(1, 0, 2).reshape(P, HC * H)
            .astype(f8))
    else:
        wv_b = np.ascontiguousarray(Wv.astype(bf))

    in_maps = []
    for b in range(B):
        xn = np.ascontiguousarray(x[b] + 2.0 * bv)
        im = {"bq": bq, "bk": bk}
        if QKDR:
            im["wq8"] = wq8
            im["wk8"] = wk8
        else:
            im["xT"] = np.ascontiguousarray(x[b].T.astype(bf))
            im["wq"] = wq_b
            im["wk"] = wk_b
        if YBF:
            im["xnb"] = xn.astype(bf)
        else:
            im["xn"] = xn
        if VDR:
            # xt8[p, k, s] = x[b].T[k*128+p, s]
            im["xt8"] = np.ascontiguousarray(
                x[b].T.reshape(HC, P, S).transpose(1, 0, 2)
                .reshape(P, HC * S).astype(f8))
            im["wv8"] = wv8
        else:
            im["wv"] = wv_b
        if affine:
            im["gam"] = np.ascontiguousarray(
                np.asarray(inputs["ln_gamma"], dtype=np.float32))
            im["bet"] = np.ascontiguousarray(
                np.asarray(inputs["ln_beta"], dtype=np.float32))
        in_maps.append(im)
    return in_maps


def run(inputs, trace=False):
    nc = _get_nc(_is_affine(inputs))
    in_maps = make_in_maps(inputs)
    res = bass_utils.run_bass_kernel_spmd(
        nc, in_maps, core_ids=list(range(B)), trace=trace
    )
    yk = "yb" if YBF else "y"
    out = np.stack(
        [np.asarray(r[yk]).astype(np.float32) for r in res.results], axis=0)
    return out, res


def kernel(**inputs) -> np.ndarray:
    out, _ = run(inputs, trace=False)
    return out

